# revision 15
# baseline (speedup 1.0000x reference)
"""KNN top-k kernel for Trainium2 (8 NeuronCores, SPMD).

Problem: seed [2, 16384, 3] queries, points [2, 16384, 3] candidates, k=16.
Output: indices of the k nearest points per query, [2, 16384, 16] int32,
matching jax.lax.top_k(-dist, k)[1] (ties -> lower index first).

Strategy (data-parallel over batch x query-quarters; within a core, m is
sharded into 512 slots of 32 with a per-slot max of -d^2, followed by a
host-side top-C slot selection + exact rescore):

  device (per core = 1 batch x 4096 queries x all 16384 points):
    - TensorE: scores g[q, m] = -d^2(q, m) exactly, via K=5 float32r
      matmuls: [2sx,2sy,2sz,-1,-|s|^2] . [px,py,pz,|p|^2,1].
      float32r runs at 1 cycle/row (4x faster than fp32) with ~fp32-like
      precision (split bf16 passes in HW).
    - Points are PRE-PERMUTED on host into member-plane order: column
      j*512 + s holds original point s*32 + j, so the 32:1 slot-max
      becomes elementwise max of 32 contiguous 512-wide planes.
    - Fold (per query-tile, 8 PSUM chunks of 2048 = 4 planes each):
        ScalarE (ACT) copies 6 chunks PSUM f32 -> SBUF bf16 (~1.2G el/s/lane)
        VectorE (DVE) zips the other 2 chunks against ACT copies
        (tensor_tensor max, PSUM+SBUF operands) and folds the remaining
        bf16 planes with 2x-mode tensor_tensor max ops down to
        A[128, 512] bf16.
    - DMA out A [4096, 512] bf16.
  host:
    - top-C slots per query by A (C=48 >> worst-case needed; a slot
      hosting one of the true top-16 has A >= -d16^2 - eps, and only a
      few slots can exceed that threshold for gaussian data).
    - exact rescore of the C*32 candidate indices with reference-identical
      f32 arithmetic, then top-k by (dist, index) - reproducing top_k tie
      semantics exactly.
"""

import numpy as np

B = 2
N = 16384          # queries per batch
M = 16384          # points per batch
D = 3
KROWS = 5          # matmul contraction rows
N_CORES = 8
Q_PER_CORE = (B * N) // N_CORES   # 4096
TILE_Q = 128
N_TILES = Q_PER_CORE // TILE_Q    # 32
FOLD = 32
SLOTS = M // FOLD                 # 512
CHUNK = 1024                      # m per PSUM buffer = 2 planes
N_CHUNKS = M // CHUNK             # 16
C_SLOTS = 48                      # host-selected candidate slots per query
USE_GPSIMD = False                # TensorTensor is not a Pool-engine opcode on trn2
FILLER = 2                        # dummy matmuls per chunk: keep the PE p-state
                                  # ramped at 2.4 GHz through drain-paced gaps

_compiled = None


def _build_bass():
    import concourse.bass as bass  # noqa: F401  (registers engine classes)
    import concourse.mybir as mybir
    import concourse.tile as tile
    from concourse import bacc

    f32 = mybir.dt.float32
    f32r = mybir.dt.float32r
    bf16 = mybir.dt.bfloat16
    mx = mybir.AluOpType.max
    nc = bacc.Bacc(None, target_bir_lowering=False)
    pts = nc.dram_tensor("pts", [KROWS, M], f32r, kind="ExternalInput")
    cfs = nc.dram_tensor("cfs", [KROWS, Q_PER_CORE], f32r, kind="ExternalInput")
    a_out = nc.dram_tensor("afold", [Q_PER_CORE, SLOTS], bf16, kind="ExternalOutput")

    with tile.TileContext(nc) as tc:
        with (
            tc.tile_pool(name="const", bufs=1) as cpool,
            tc.tile_pool(name="s16", bufs=6) as spool,
            tc.tile_pool(name="sz32", bufs=4) as s32pool,
            tc.tile_pool(name="zp", bufs=8) as zpool,
            tc.tile_pool(name="wp", bufs=3) as wpool,
            tc.tile_pool(name="up", bufs=6) as upool,
            tc.tile_pool(name="vp", bufs=3) as vpool,
            tc.tile_pool(name="tp", bufs=2) as tpool,
            tc.tile_pool(name="ap", bufs=3) as apool,
            tc.tile_pool(name="psum", bufs=3, space="PSUM") as ppool,
            tc.tile_pool(name="pdummy", bufs=1, space="PSUM") as dpool,
        ):
            pts_sb = cpool.tile([KROWS, M], f32r)
            nc.sync.dma_start(pts_sb[:], pts[:])
            cfs_sb = cpool.tile([KROWS, Q_PER_CORE], f32r)
            nc.sync.dma_start(cfs_sb[:], cfs[:])

            eng2 = nc.gpsimd if USE_GPSIMD else nc.vector
            F32COPY = (0, 1, 4, 5, 8, 9)      # ACT f32 copies (zip partners)
            ZIP = (2, 3, 6, 7, 10, 11)        # DVE ttmax(PSUM chunk, partner)
            for t in range(N_TILES):
                lhsT = cfs_sb[:, t * TILE_Q:(t + 1) * TILE_Q]
                sb = {}     # chunk idx -> SBUF copy (ACT)
                z = []      # DVE zip outputs (PSUM x SBUF max)
                u = []      # second-level folds
                for c in range(N_CHUNKS):
                    pc = ppool.tile([TILE_Q, CHUNK], f32, tag="ps")
                    for j in range(CHUNK // 512):
                        off = c * CHUNK + j * 512
                        nc.tensor.matmul(
                            pc[:, j * 512:(j + 1) * 512],
                            lhsT,
                            pts_sb[:, off:off + 512],
                        )
                    for _ in range(FILLER):
                        dt_ = dpool.tile([TILE_Q, 512], f32, tag="d")
                        nc.tensor.matmul(dt_[:], lhsT, pts_sb[:, 0:512])
                    if c in ZIP:
                        # DVE drains this chunk, folding in an earlier
                        # ACT f32 copy (f32 x f32 -> bf16).
                        zt = zpool.tile([TILE_Q, CHUNK], bf16, tag="z")
                        nc.vector.tensor_tensor(zt[:], pc[:], sb[c - 2][:], op=mx)
                        z.append(zt)
                        if len(z) % 2 == 0:
                            ut = upool.tile([TILE_Q, CHUNK], bf16, tag="u")
                            nc.vector.tensor_tensor(
                                ut[:], z[-2][:], z[-1][:], op=mx)
                            u.append(ut)
                    elif c in F32COPY:
                        st = s32pool.tile([TILE_Q, CHUNK], f32, tag="s32")
                        nc.scalar.copy(st[:], pc[:])
                        sb[c] = st
                    else:
                        st = spool.tile([TILE_Q, CHUNK], bf16, tag="s")
                        nc.scalar.copy(st[:], pc[:])
                        sb[c] = st
                # fold the four ACT bf16 chunks (2x mode)
                w0 = wpool.tile([TILE_Q, CHUNK], bf16, tag="w")
                nc.vector.tensor_tensor(w0[:], sb[12][:], sb[13][:], op=mx)
                w1 = wpool.tile([TILE_Q, CHUNK], bf16, tag="w")
                nc.vector.tensor_tensor(w1[:], sb[14][:], sb[15][:], op=mx)
                u3 = upool.tile([TILE_Q, CHUNK], bf16, tag="u")
                nc.vector.tensor_tensor(u3[:], w0[:], w1[:], op=mx)
                u.append(u3)
                # final pyramid (8 planes -> 1) on the second engine
                v0 = vpool.tile([TILE_Q, CHUNK], bf16, tag="v")
                eng2.tensor_tensor(v0[:], u[0][:], u[1][:], op=mx)
                v1 = vpool.tile([TILE_Q, CHUNK], bf16, tag="v")
                eng2.tensor_tensor(v1[:], u[2][:], u[3][:], op=mx)
                tt = tpool.tile([TILE_Q, CHUNK], bf16, tag="t")
                eng2.tensor_tensor(tt[:], v0[:], v1[:], op=mx)
                at = apool.tile([TILE_Q, SLOTS], bf16, tag="a")
                eng2.tensor_tensor(at[:], tt[:, 0:512], tt[:, 512:1024], op=mx)
                nc.sync.dma_start(a_out[t * TILE_Q:(t + 1) * TILE_Q, :], at[:])
    nc.compile()
    return nc


def _make_in_maps(seed_f, points_f):
    in_maps = []
    for core in range(N_CORES):
        b = core // (N_CORES // B)
        qq = core % (N_CORES // B)
        s = seed_f[b, qq * Q_PER_CORE:(qq + 1) * Q_PER_CORE]   # [4096, 3]
        p = points_f[b]                                         # [16384, 3]
        pn2 = p[:, 0] * p[:, 0] + p[:, 1] * p[:, 1] + p[:, 2] * p[:, 2]
        pts_in = np.empty((KROWS, M), np.float32)
        pts_in[0] = p[:, 0]
        pts_in[1] = p[:, 1]
        pts_in[2] = p[:, 2]
        pts_in[3] = pn2
        pts_in[4] = 1.0
        # member-plane permutation: column j*SLOTS + s <- point s*FOLD + j
        pts_in = np.ascontiguousarray(
            pts_in.reshape(KROWS, SLOTS, FOLD).transpose(0, 2, 1).reshape(KROWS, M))
        sn2 = s[:, 0] * s[:, 0] + s[:, 1] * s[:, 1] + s[:, 2] * s[:, 2]
        cfs_in = np.empty((KROWS, Q_PER_CORE), np.float32)
        cfs_in[0] = 2.0 * s[:, 0]
        cfs_in[1] = 2.0 * s[:, 1]
        cfs_in[2] = 2.0 * s[:, 2]
        cfs_in[3] = -1.0
        cfs_in[4] = -sn2
        in_maps.append({"pts": pts_in, "cfs": cfs_in})
    return in_maps


def _device_fold(seed_f, points_f):
    """Run the SPMD bass kernel; returns A folds [B, N, SLOTS] f32 (= -d^2 max)."""
    from concourse.bass_utils import run_bass_kernel_spmd

    global _compiled
    if _compiled is None:
        _compiled = _build_bass()
    nc = _compiled

    in_maps = _make_in_maps(seed_f, points_f)
    res = run_bass_kernel_spmd(nc, in_maps, core_ids=list(range(N_CORES)))
    a = np.empty((B, N, SLOTS), np.float32)
    for core in range(N_CORES):
        b = core // (N_CORES // B)
        qq = core % (N_CORES // B)
        a[b, qq * Q_PER_CORE:(qq + 1) * Q_PER_CORE] = np.asarray(
            res.results[core]["afold"], dtype=np.float32)
    return a


def _host_topk(seed_f, points_f, a, k):
    """Exact top-k from fold maxima: select top-C slots, rescore exactly."""
    c_slots = max(C_SLOTS, int(k) + 24)
    out = np.empty((B, N, int(k)), np.int32)
    sub = np.arange(FOLD, dtype=np.int64)
    for b in range(B):
        p = points_f[b]
        px, py, pz = p[:, 0], p[:, 1], p[:, 2]
        for q0 in range(0, N, 2048):
            q1 = min(q0 + 2048, N)
            ab = a[b, q0:q1]
            s = seed_f[b, q0:q1]
            # top-C slots per query (order within C irrelevant)
            sel = np.argpartition(-ab, c_slots - 1, axis=1)[:, :c_slots]
            cand = (sel[:, :, None].astype(np.int64) * FOLD + sub).reshape(q1 - q0, -1)
            # exact reference-style f32 distances
            dx = s[:, 0:1] - px[cand]
            dy = s[:, 1:2] - py[cand]
            dz = s[:, 2:3] - pz[cand]
            dist = dx * dx + dy * dy
            dist += dz * dz
            # top-k by (dist, index): stable mergesort on dist of
            # index-ascending-sorted candidates reproduces top_k ties
            ordc = np.argsort(cand, axis=1, kind="stable")
            cand_s = np.take_along_axis(cand, ordc, axis=1)
            dist_s = np.take_along_axis(dist, ordc, axis=1)
            pick = np.argsort(dist_s, axis=1, kind="stable")[:, :int(k)]
            out[b, q0:q1] = np.take_along_axis(cand_s, pick, axis=1).astype(np.int32)
    return out


def kernel(seed, points, k):
    seed_f = np.ascontiguousarray(np.asarray(seed), dtype=np.float32)
    points_f = np.ascontiguousarray(np.asarray(points), dtype=np.float32)
    kk = int(k)
    assert seed_f.shape == (B, N, D) and points_f.shape == (B, M, D)
    a = _device_fold(seed_f, points_f)
    return _host_topk(seed_f, points_f, a, kk)


# revision 16
# speedup vs baseline: 1.8832x; 1.8832x over previous
"""KNN top-k kernel for Trainium2 (8 NeuronCores, SPMD).

Problem: seed [2, 16384, 3] queries, points [2, 16384, 3] candidates, k=16.
Output: indices of the k nearest points per query, [2, 16384, 16] int32,
matching jax.lax.top_k(-dist, k)[1] (ties -> lower index first).

Strategy (data-parallel over batch x query-quarters; within a core, m is
sharded into 512 slots of 32 with a per-slot max of -d^2, followed by a
host-side top-C slot selection + exact rescore):

  device (per core = 1 batch x 4096 queries x all 16384 points):
    - TensorE: scores g[q, m] = -d^2(q, m) exactly, via K=5 float32r
      matmuls: [2sx,2sy,2sz,-1,-|s|^2] . [px,py,pz,|p|^2,1].
      float32r runs at 1 cycle/row (4x faster than fp32) with ~fp32-like
      precision (split bf16 passes in HW).
    - Points are PRE-PERMUTED on host into member-plane order: column
      j*512 + s holds original point s*32 + j, so the 32:1 slot-max
      becomes elementwise max of 32 contiguous 512-wide planes.
    - Fold (per query-tile, 8 PSUM chunks of 2048 = 4 planes each):
        ScalarE (ACT) copies 6 chunks PSUM f32 -> SBUF bf16 (~1.2G el/s/lane)
        VectorE (DVE) zips the other 2 chunks against ACT copies
        (tensor_tensor max, PSUM+SBUF operands) and folds the remaining
        bf16 planes with 2x-mode tensor_tensor max ops down to
        A[128, 512] bf16.
    - DMA out A [4096, 512] bf16.
  host:
    - top-C slots per query by A (C=48 >> worst-case needed; a slot
      hosting one of the true top-16 has A >= -d16^2 - eps, and only a
      few slots can exceed that threshold for gaussian data).
    - exact rescore of the C*32 candidate indices with reference-identical
      f32 arithmetic, then top-k by (dist, index) - reproducing top_k tie
      semantics exactly.
"""

import ml_dtypes
import numpy as np

B = 2
N = 16384          # queries per batch
M = 16384          # points per batch
D = 3
KROWS = 15         # matmul contraction rows: bf16 hi/lo split
                   # [c_hi, c_hi, c_lo] . [p_hi, p_lo, p_hi]
N_CORES = 8
Q_PER_CORE = (B * N) // N_CORES   # 4096
TILE_Q = 128
N_TILES = Q_PER_CORE // TILE_Q    # 32
FOLD = 32
SLOTS = M // FOLD                 # 512
CHUNK = 1024                      # m per PSUM buffer = 2 planes
N_CHUNKS = M // CHUNK             # 16
C_SLOTS = 48                      # host-selected candidate slots per query
USE_GPSIMD = False                # TensorTensor is not a Pool-engine opcode on trn2
FILLER = 0                        # PE runs 1 cyc/row at bf16 regardless of gaps;
                                  # filler matmuls only added work (measured)

_compiled = None


def _build_bass():
    import concourse.bass as bass  # noqa: F401  (registers engine classes)
    import concourse.mybir as mybir
    import concourse.tile as tile
    from concourse import bacc

    f32 = mybir.dt.float32
    bf16 = mybir.dt.bfloat16
    mx = mybir.AluOpType.max
    nc = bacc.Bacc(None, target_bir_lowering=False)
    pts = nc.dram_tensor("pts", [KROWS, M], bf16, kind="ExternalInput")
    cfs = nc.dram_tensor("cfs", [KROWS, Q_PER_CORE], bf16, kind="ExternalInput")
    a_out = nc.dram_tensor("afold", [Q_PER_CORE, SLOTS], bf16, kind="ExternalOutput")

    with tile.TileContext(nc) as tc:
        with (
            tc.tile_pool(name="const", bufs=1) as cpool,
            tc.tile_pool(name="s16", bufs=6) as spool,
            tc.tile_pool(name="sz32", bufs=4) as s32pool,
            tc.tile_pool(name="zp", bufs=8) as zpool,
            tc.tile_pool(name="wp", bufs=3) as wpool,
            tc.tile_pool(name="up", bufs=6) as upool,
            tc.tile_pool(name="vp", bufs=3) as vpool,
            tc.tile_pool(name="tp", bufs=2) as tpool,
            tc.tile_pool(name="ap", bufs=3) as apool,
            tc.tile_pool(name="psum", bufs=4, space="PSUM") as ppool,
        ):
            pts_sb = cpool.tile([KROWS, M], bf16)
            nc.sync.dma_start(pts_sb[:], pts[:])
            cfs_sb = cpool.tile([KROWS, Q_PER_CORE], bf16)
            nc.sync.dma_start(cfs_sb[:], cfs[:])

            eng2 = nc.gpsimd if USE_GPSIMD else nc.vector
            F32COPY = (0, 1, 4, 5, 8, 9)      # ACT f32 copies (zip partners)
            ZIP = (2, 3, 6, 7, 10, 11)        # DVE ttmax(PSUM chunk, partner)
            for t in range(N_TILES):
                lhsT = cfs_sb[:, t * TILE_Q:(t + 1) * TILE_Q]
                sb = {}     # chunk idx -> SBUF copy (ACT)
                z = []      # DVE zip outputs (PSUM x SBUF max)
                u = []      # second-level folds
                for c in range(N_CHUNKS):
                    pc = ppool.tile([TILE_Q, CHUNK], f32, tag="ps")
                    for j in range(CHUNK // 512):
                        off = c * CHUNK + j * 512
                        nc.tensor.matmul(
                            pc[:, j * 512:(j + 1) * 512],
                            lhsT,
                            pts_sb[:, off:off + 512],
                        )
                    if c in ZIP:
                        # DVE drains this chunk, folding in an earlier
                        # ACT f32 copy (f32 x f32 -> bf16).
                        zt = zpool.tile([TILE_Q, CHUNK], bf16, tag="z")
                        nc.vector.tensor_tensor(zt[:], pc[:], sb[c - 2][:], op=mx)
                        z.append(zt)
                        if len(z) % 2 == 0:
                            ut = upool.tile([TILE_Q, CHUNK], bf16, tag="u")
                            nc.vector.tensor_tensor(
                                ut[:], z[-2][:], z[-1][:], op=mx)
                            u.append(ut)
                    elif c in F32COPY:
                        st = s32pool.tile([TILE_Q, CHUNK], f32, tag="s32")
                        nc.scalar.copy(st[:], pc[:])
                        sb[c] = st
                    else:
                        st = spool.tile([TILE_Q, CHUNK], bf16, tag="s")
                        nc.scalar.copy(st[:], pc[:])
                        sb[c] = st
                # fold the four ACT bf16 chunks (2x mode)
                w0 = wpool.tile([TILE_Q, CHUNK], bf16, tag="w")
                nc.vector.tensor_tensor(w0[:], sb[12][:], sb[13][:], op=mx)
                w1 = wpool.tile([TILE_Q, CHUNK], bf16, tag="w")
                nc.vector.tensor_tensor(w1[:], sb[14][:], sb[15][:], op=mx)
                u3 = upool.tile([TILE_Q, CHUNK], bf16, tag="u")
                nc.vector.tensor_tensor(u3[:], w0[:], w1[:], op=mx)
                u.append(u3)
                # final pyramid (8 planes -> 1) on the second engine
                v0 = vpool.tile([TILE_Q, CHUNK], bf16, tag="v")
                eng2.tensor_tensor(v0[:], u[0][:], u[1][:], op=mx)
                v1 = vpool.tile([TILE_Q, CHUNK], bf16, tag="v")
                eng2.tensor_tensor(v1[:], u[2][:], u[3][:], op=mx)
                tt = tpool.tile([TILE_Q, CHUNK], bf16, tag="t")
                eng2.tensor_tensor(tt[:], v0[:], v1[:], op=mx)
                at = apool.tile([TILE_Q, SLOTS], bf16, tag="a")
                eng2.tensor_tensor(at[:], tt[:, 0:512], tt[:, 512:1024], op=mx)
                nc.sync.dma_start(a_out[t * TILE_Q:(t + 1) * TILE_Q, :], at[:])
    nc.compile()
    return nc


def _make_in_maps(seed_f, points_f):
    in_maps = []
    for core in range(N_CORES):
        b = core // (N_CORES // B)
        qq = core % (N_CORES // B)
        s = seed_f[b, qq * Q_PER_CORE:(qq + 1) * Q_PER_CORE]   # [4096, 3]
        p = points_f[b]                                         # [16384, 3]
        pn2 = p[:, 0] * p[:, 0] + p[:, 1] * p[:, 1] + p[:, 2] * p[:, 2]
        pv = np.empty((5, M), np.float32)
        pv[0] = p[:, 0]
        pv[1] = p[:, 1]
        pv[2] = p[:, 2]
        pv[3] = pn2
        pv[4] = 1.0
        sn2 = s[:, 0] * s[:, 0] + s[:, 1] * s[:, 1] + s[:, 2] * s[:, 2]
        cv = np.empty((5, Q_PER_CORE), np.float32)
        cv[0] = 2.0 * s[:, 0]
        cv[1] = 2.0 * s[:, 1]
        cv[2] = 2.0 * s[:, 2]
        cv[3] = -1.0
        cv[4] = -sn2
        # bf16 hi/lo split: c.p ~= chi.phi + chi.plo + clo.phi  (K=15)
        bf = ml_dtypes.bfloat16
        phi = pv.astype(bf)
        plo = (pv - phi.astype(np.float32)).astype(bf)
        chi = cv.astype(bf)
        clo = (cv - chi.astype(np.float32)).astype(bf)
        pts_in = np.concatenate([phi, plo, phi], axis=0)
        cfs_in = np.concatenate([chi, chi, clo], axis=0)
        # member-plane permutation: column j*SLOTS + s <- point s*FOLD + j
        pts_in = np.ascontiguousarray(
            pts_in.reshape(KROWS, SLOTS, FOLD).transpose(0, 2, 1).reshape(KROWS, M))
        in_maps.append({"pts": pts_in, "cfs": np.ascontiguousarray(cfs_in)})
    return in_maps


def _device_fold(seed_f, points_f):
    """Run the SPMD bass kernel; returns A folds [B, N, SLOTS] f32 (= -d^2 max)."""
    from concourse.bass_utils import run_bass_kernel_spmd

    global _compiled
    if _compiled is None:
        _compiled = _build_bass()
    nc = _compiled

    in_maps = _make_in_maps(seed_f, points_f)
    res = run_bass_kernel_spmd(nc, in_maps, core_ids=list(range(N_CORES)))
    a = np.empty((B, N, SLOTS), np.float32)
    for core in range(N_CORES):
        b = core // (N_CORES // B)
        qq = core % (N_CORES // B)
        a[b, qq * Q_PER_CORE:(qq + 1) * Q_PER_CORE] = np.asarray(
            res.results[core]["afold"], dtype=np.float32)
    return a


def _host_topk(seed_f, points_f, a, k):
    """Exact top-k from fold maxima: select top-C slots, rescore exactly."""
    c_slots = max(C_SLOTS, int(k) + 24)
    out = np.empty((B, N, int(k)), np.int32)
    sub = np.arange(FOLD, dtype=np.int64)
    for b in range(B):
        p = points_f[b]
        px, py, pz = p[:, 0], p[:, 1], p[:, 2]
        for q0 in range(0, N, 2048):
            q1 = min(q0 + 2048, N)
            ab = a[b, q0:q1]
            s = seed_f[b, q0:q1]
            # top-C slots per query (order within C irrelevant)
            sel = np.argpartition(-ab, c_slots - 1, axis=1)[:, :c_slots]
            cand = (sel[:, :, None].astype(np.int64) * FOLD + sub).reshape(q1 - q0, -1)
            # exact reference-style f32 distances
            dx = s[:, 0:1] - px[cand]
            dy = s[:, 1:2] - py[cand]
            dz = s[:, 2:3] - pz[cand]
            dist = dx * dx + dy * dy
            dist += dz * dz
            # top-k by (dist, index): stable mergesort on dist of
            # index-ascending-sorted candidates reproduces top_k ties
            ordc = np.argsort(cand, axis=1, kind="stable")
            cand_s = np.take_along_axis(cand, ordc, axis=1)
            dist_s = np.take_along_axis(dist, ordc, axis=1)
            pick = np.argsort(dist_s, axis=1, kind="stable")[:, :int(k)]
            out[b, q0:q1] = np.take_along_axis(cand_s, pick, axis=1).astype(np.int32)
    return out


def kernel(seed, points, k):
    seed_f = np.ascontiguousarray(np.asarray(seed), dtype=np.float32)
    points_f = np.ascontiguousarray(np.asarray(points), dtype=np.float32)
    kk = int(k)
    assert seed_f.shape == (B, N, D) and points_f.shape == (B, M, D)
    a = _device_fold(seed_f, points_f)
    return _host_topk(seed_f, points_f, a, kk)
